# revision 5
# baseline (speedup 1.0000x reference)
"""Dice-loss (segment_reduce) kernel for 8 Trainium2 NeuronCores.

Full inputs: input (4,5,128,128,128) f32, target (4,128,128,128) int64.
Output: scalar mean dice, shape (1,), f32 — matches the jax reference.

Sharding: 8 cores = 4 batches x 2 spatial halves. Each core computes, for
its 1,048,576 positions, per-class counts for classes 1..4:
  P_c = #(x_c == max over classes)        (argmax one-hot, ties overcount)
  T_c = #(target == c)
  I_c = #((x_c == max) and target == c)
The tiny (b,c) count tensors are gathered to the host, which forms
dice = (2I+eps)/(P+T+eps) and the final mean.

Per core the device streams 21 MiB (x: 20 MiB f32, target: 1 MiB int8) and
balances compare/count work across GpSimd (max chain), ScalarE (target
one-hot via Abs/Relu with fused accumulate) and VectorE (argmax one-hot +
intersection, fused accumulate).
"""

import sys

sys.path.insert(0, "/opt/trn_rl_repo")

import numpy as np
import concourse.bass as bass
import concourse.mybir as mybir
from concourse.tile import TileContext
from concourse.bass_utils import run_bass_kernel_spmd

F32 = mybir.dt.float32
BF16 = mybir.dt.bfloat16
I8 = mybir.dt.int8
Alu = mybir.AluOpType
Act = mybir.ActivationFunctionType

B, C = 4, 5
N = 128 * 128 * 128          # spatial positions per batch
NCORES = 8
HALF = N // 2                # positions per core
P = 128                      # SBUF partitions
M = 1024                     # free-dim elements per chunk
NCH = HALF // (P * M)        # chunks per core (8)
EPS = 1e-5

_prog_cache = {}


def _legalize_waits(nc):
    """Split multi-wait instructions: this walrus build's codegen allows only
    one embedded sync-wait per instruction ("Too many sync wait commands").
    Move extra waits onto standalone EventSemaphore instructions inserted
    just before, on the same engine queue — semantically identical."""
    n_new = 0
    for bb in nc.main_func.blocks:
        insts = list(bb.instructions)
        out = []
        changed = False
        for ins in insts:
            si = ins.sync_info
            waits = list(si.on_wait) if si and si.on_wait else []
            if len(waits) > 1:
                for w in waits[:-1]:
                    ev = mybir.InstEventSemaphore(
                        name=f"legalw-{n_new}", ins=[], outs=[]
                    )
                    n_new += 1
                    ev.engine = ins.engine
                    ev.sync_info = mybir.SyncInfo(on_wait=[w], on_update=[])
                    nc.register_instruction(ev)
                    out.append(ev)
                ins.sync_info = mybir.SyncInfo(
                    on_wait=[waits[-1]], on_update=list(si.on_update or [])
                )
                changed = True
            out.append(ins)
        if changed:
            live = bb.instructions
            live.clear()
            live.extend(out)
    return n_new


def _build_program():
    nc = bass.Bass()

    # Register activation bias constants so ACT ops carry no extra runtime
    # dependency (same pattern as Bass.__init__'s const APs).
    for v in (-1.0, -2.0, -3.0, -4.0):
        t_ = nc.alloc_sbuf_tensor(f"const-f32-{v}", [P, 1], F32)
        nc.gpsimd.memset(t_.ap(), v)
        nc.const_aps.aps[(F32, v)] = t_.ap()
    nc.all_engine_barrier()

    x = nc.dram_tensor("x", [C, HALF], F32, kind="ExternalInput")
    t = nc.dram_tensor("t", [HALF], I8, kind="ExternalInput")
    yp = nc.dram_tensor("yp", [P, 4 * NCH], F32, kind="ExternalOutput")
    yt = nc.dram_tensor("yt", [P, 4 * NCH], F32, kind="ExternalOutput")
    yi = nc.dram_tensor("yi", [P, 4 * NCH], F32, kind="ExternalOutput")

    # chunk ch covers positions [ch*P*M, (ch+1)*P*M); element (ch, p, c, f)
    xr = x[:].rearrange("c (ch p f) -> ch p c f", ch=NCH, p=P, f=M)
    tr = t[:].rearrange("(ch p f) -> ch p f", ch=NCH, p=P, f=M)

    with TileContext(nc) as tc:
        with (
            tc.tile_pool(name="xin", bufs=2) as pool_x,
            tc.tile_pool(name="tin", bufs=2) as pool_t,
            tc.tile_pool(name="work", bufs=2) as pool_w,
            tc.tile_pool(name="accs", bufs=1) as pool_a,
        ):
            accP = pool_a.tile([P, 4 * NCH], F32)
            accT = pool_a.tile([P, 4 * NCH], F32)
            accI = pool_a.tile([P, 4 * NCH], F32)

            for ch in range(NCH):
                xt = pool_x.tile([P, C, M], F32, tag="xt")
                tt = pool_t.tile([P, M], I8, tag="tt")
                nc.sync.dma_start(out=xt[:], in_=xr[ch])
                nc.sync.dma_start(out=tt[:], in_=tr[ch])

                # VectorE: max over the 5 classes (tree; Pool can't do TT in
                # this walrus build)
                ma = pool_w.tile([P, M], F32, tag="ma")
                mb = pool_w.tile([P, M], F32, tag="mb")
                mc_ = pool_w.tile([P, M], F32, tag="mc")
                mx = pool_w.tile([P, M], F32, tag="mx")
                nc.vector.tensor_tensor(out=ma[:], in0=xt[:, 0, :], in1=xt[:, 1, :], op=Alu.max)
                nc.vector.tensor_tensor(out=mb[:], in0=xt[:, 2, :], in1=xt[:, 3, :], op=Alu.max)
                nc.vector.tensor_tensor(out=mc_[:], in0=ma[:], in1=mb[:], op=Alu.max)
                nc.vector.tensor_tensor(out=mx[:], in0=mc_[:], in1=xt[:, 4, :], op=Alu.max)

                # ScalarE: target one-hot teq_c = Relu(1 - |t - c|), counts
                # accumulate into accT.
                teqs = []
                for c in range(1, C):
                    ab = pool_w.tile([P, M], BF16, tag=f"ab{c}")
                    nc.scalar.activation(out=ab[:], in_=tt[:], func=Act.Abs, bias=float(-c), scale=1.0)
                    teqs.append(ab)
                for c in range(1, C):
                    te = pool_w.tile([P, M], BF16, tag=f"te{c}")
                    nc.scalar.activation(
                        out=te[:], in_=teqs[c - 1][:], func=Act.Relu, bias=1.0, scale=-1.0,
                        accum_out=accT[:, ch * 4 + c - 1 : ch * 4 + c],
                    )
                    teqs[c - 1] = te

                # VectorE: eq_c = (x_c >= max) with count; then intersection
                eqs = []
                for c in range(1, C):
                    eq = pool_w.tile([P, M], BF16, tag=f"eq{c}")
                    col = ch * 4 + c - 1
                    nc.vector.scalar_tensor_tensor(
                        out=eq[:], in0=xt[:, c, :], scalar=0.0, in1=mx[:],
                        op0=Alu.add, op1=Alu.is_ge,
                        accum_out=accP[:, col : col + 1],
                    )
                    eqs.append(eq)
                junk = pool_w.tile([P, M], BF16, tag="junk")
                for c in range(1, C):
                    col = ch * 4 + c - 1
                    nc.vector.scalar_tensor_tensor(
                        out=junk[:], in0=eqs[c - 1][:], scalar=1.0, in1=teqs[c - 1][:],
                        op0=Alu.mult, op1=Alu.mult,
                        accum_out=accI[:, col : col + 1],
                    )

            nc.sync.dma_start(out=yp[:], in_=accP[:])
            nc.sync.dma_start(out=yt[:], in_=accT[:])
            nc.sync.dma_start(out=yi[:], in_=accI[:])

    _legalize_waits(nc)
    return nc


def _get_program():
    if "nc" not in _prog_cache:
        _prog_cache["nc"] = _build_program()
    return _prog_cache["nc"]


def _run(input, target, trace=False, trace_kwargs=None):
    inp = np.asarray(input)
    tgt = np.asarray(target)
    assert inp.shape == (B, C, 128, 128, 128), inp.shape
    assert tgt.shape == (B, 128, 128, 128), tgt.shape

    inp_r = inp.reshape(B, C, N)
    tgt_r = tgt.reshape(B, N)

    in_maps = []
    for core in range(NCORES):
        b, h = core // 2, core % 2
        xs = np.ascontiguousarray(inp_r[b, :, h * HALF : (h + 1) * HALF])
        ts_ = tgt_r[b, h * HALF : (h + 1) * HALF].astype(np.int8)
        in_maps.append({"x": xs, "t": ts_})

    nc = _get_program()
    kw = {}
    if trace:
        kw["trace"] = True
        if trace_kwargs:
            kw.update(trace_kwargs)
    res = run_bass_kernel_spmd(nc, in_maps, list(range(NCORES)), **kw)

    # host combine: per (batch, class) counts from the two half-cores
    Pc = np.zeros((B, C), np.float64)
    Tc = np.zeros((B, C), np.float64)
    Ic = np.zeros((B, C), np.float64)
    for core in range(NCORES):
        b = core // 2
        r = res.results[core]
        for c in range(1, C):
            cols = slice(c - 1, 4 * NCH, 4)
            Pc[b, c] += r["yp"][:, cols].sum()
            Tc[b, c] += r["yt"][:, cols].sum()
            Ic[b, c] += r["yi"][:, cols].sum()

    inter = Ic[:, 1:].astype(np.float32)
    union = (Pc[:, 1:] + Tc[:, 1:]).astype(np.float32)
    dice = (2.0 * inter + np.float32(EPS)) / (union + np.float32(EPS))
    out = np.array([dice.mean(dtype=np.float32)], dtype=np.float32)
    return out, res


def kernel(input, target):
    out, _ = _run(input, target, trace=False)
    return out
